# revision 1
# baseline (speedup 1.0000x reference)
"""Low-rank (CPD) 3D conv kernel for Trainium2, SPMD across 8 NeuronCores.

Math (per reference):
  y[r,h,w,d]  = sum_c U_c_in[c,r] * x[c,h,w,d]
  y           = conv_h(conv_w(conv_d-separable 3-tap, per-rank taps U_k*))
  out[c,...]  = sum_r U_c_out[r,c] * z[r,...] + bias[c]

Distribution: data-parallel split of H (64) into 8 slabs of 8 planes; each
core reads its slab plus one halo plane on each side (zero at global edges)
and computes its output slab independently. No collectives.

Per-core pipeline (streamed over the 8 output planes):
  - mm1 with conv_h folded: 3 weight matrices W_k = U_c_in * U_kh[k] (host
    precomputed, bf16); PSUM accumulation over 2 c-tiles x 3 h-taps.
  - PSUM drain on ScalarE, casting to bf16 and de-interleaving d into
    (even,odd) halves per w-line so the d-shifts below stay 4B-aligned.
  - conv_w on VectorE: per-partition scale (tensor_scalar) + 2 fused
    scale-add passes (scalar_tensor_tensor) with +-1 w-line shifts.
  - conv_d on VectorE: same, operating across the even/odd halves.
  - mm2: lhsT = U_c_out (bf16), accumulate 2 r-tiles.
  - PSUM drain on ScalarE with per-partition bias add, re-interleaving d,
    f32 output.
"""

import numpy as np
import ml_dtypes

BF16 = ml_dtypes.bfloat16

# Problem constants (hardcoded per contest contract)
C = 256   # input channels
R = 256   # rank
CO = 256  # output channels
S = 64    # spatial extent (cube)
NCORES = 8
HP = S // NCORES          # output planes per core (8)
HS = HP + 2               # slab planes incl. halo (10)
PLANE = S * S             # 4096 elements per (w,d) plane
NCH = PLANE // 512        # 512-column matmul chunks per plane (8)

_cache = {}


def _build_program(hp=HP, wl=S):
    """Build and compile the per-core Bass program (identical on all cores).

    hp: output planes per core; wl: w-lines per plane (64 in production).
    """
    import concourse.bass as bass
    import concourse.mybir as mybir
    import concourse.tile as tile
    from concourse import bacc

    HS, PLANE, NCH = hp + 2, wl * 64, (wl * 64) // 512
    HP_ = hp

    fp32 = mybir.dt.float32
    bf16 = mybir.dt.bfloat16

    nc = bacc.Bacc("TRN2", target_bir_lowering=False, debug=False,
                   num_devices=NCORES)

    # DRAM tensors (names are the in_map keys)
    x_d = nc.dram_tensor("xs", [2, 128, HS, PLANE], bf16, kind="ExternalInput").ap()
    wkh_d = nc.dram_tensor("wkh", [3, 2, 2, 128, 128], bf16, kind="ExternalInput").ap()
    uco_d = nc.dram_tensor("uco", [2, 2, 128, 128], bf16, kind="ExternalInput").ap()
    ukw_d = nc.dram_tensor("ukw", [2, 128, 3], fp32, kind="ExternalInput").ap()
    ukd_d = nc.dram_tensor("ukd", [2, 128, 3], fp32, kind="ExternalInput").ap()
    bias_d = nc.dram_tensor("bias_t", [2, 128, 1], fp32, kind="ExternalInput").ap()
    out_d = nc.dram_tensor("out", [2, 128, HP_, PLANE], fp32, kind="ExternalOutput").ap()

    mult = mybir.AluOpType.mult
    add = mybir.AluOpType.add
    ident = mybir.ActivationFunctionType.Identity

    with tile.TileContext(nc) as tc:
        consts = tc.alloc_tile_pool(name="consts", bufs=1)
        xpool = tc.alloc_tile_pool(name="x", bufs=8)
        ypool = tc.alloc_tile_pool(name="y", bufs=3)
        tpool = tc.alloc_tile_pool(name="tmp", bufs=4)
        zpool = tc.alloc_tile_pool(name="z", bufs=2)
        zdpool = tc.alloc_tile_pool(name="zd", bufs=2)
        opool = tc.alloc_tile_pool(name="osb", bufs=2)
        ps1 = tc.alloc_tile_pool(name="ps1", bufs=2, space="PSUM")
        ps2 = tc.alloc_tile_pool(name="ps2", bufs=2, space="PSUM")

        # ---- constants ----
        wkh = [[[consts.tile([128, 128], bf16, name=f"wkh{k}{ct}{rt}", tag=f"wkh{k}{ct}{rt}")
                 for rt in range(2)] for ct in range(2)] for k in range(3)]
        for k in range(3):
            for ct in range(2):
                for rt in range(2):
                    nc.sync.dma_start(out=wkh[k][ct][rt], in_=wkh_d[k, ct, rt])
        uco = [[consts.tile([128, 128], bf16, name=f"uco{rt}{co}", tag=f"uco{rt}{co}")
                for co in range(2)] for rt in range(2)]
        for rt in range(2):
            for co in range(2):
                nc.sync.dma_start(out=uco[rt][co], in_=uco_d[rt, co])
        ukw = [consts.tile([128, 3], fp32, name=f"ukw{rt}", tag=f"ukw{rt}") for rt in range(2)]
        ukd = [consts.tile([128, 3], fp32, name=f"ukd{rt}", tag=f"ukd{rt}") for rt in range(2)]
        bia = [consts.tile([128, 1], fp32, name=f"bias{co}", tag=f"bias{co}") for co in range(2)]
        for rt in range(2):
            nc.sync.dma_start(out=ukw[rt], in_=ukw_d[rt])
            nc.sync.dma_start(out=ukd[rt], in_=ukd_d[rt])
        for co in range(2):
            nc.sync.dma_start(out=bia[co], in_=bias_d[co])

        # ---- x plane streaming ----
        xt = {}

        def get_x(p, ct):
            if (p, ct) not in xt:
                t = xpool.tile([128, PLANE], bf16, name="xplane", tag="xplane")
                nc.sync.dma_start(out=t, in_=x_d[ct, :, p, :])
                xt[(p, ct)] = t
            return xt[(p, ct)]

        NQ = PLANE // 1024  # 1024-wide psum tiles per plane

        for h in range(HP_):
            y = []
            t0s = []
            for rt in range(2):
                # --- mm1 + conv_h fold (PSUM 1024-tiles, 512 matmul halves) ---
                ysb = ypool.tile([128, PLANE], bf16, name="ysb", tag="y")
                t0 = tpool.tile([128, PLANE], bf16, name="t0t", tag="tmp")
                for q in range(NQ):
                    pt = ps1.tile([128, 1024], fp32, name="pt1", tag="ps1")
                    for half in range(2):
                        first = True
                        for k in range(3):
                            for ct in range(2):
                                nc.tensor.matmul(
                                    pt[:, half * 512:(half + 1) * 512],
                                    wkh[k][ct][rt],
                                    get_x(h + k, ct)[:, q * 1024 + half * 512:
                                                     q * 1024 + (half + 1) * 512],
                                    start=first,
                                    stop=(k == 2 and ct == 1),
                                )
                                first = False
                    # drains: f32 PSUM -> bf16 SBUF, de-interleave d.
                    # plain y (ACT) + U_kw[0]-scaled t0 (ACT)
                    src = pt.rearrange("p (w j s) -> p w s j", j=32, s=2)
                    dst = ysb.rearrange("p (w s j) -> p w s j", s=2, j=32)[
                        :, q * 16:(q + 1) * 16]
                    nc.scalar.copy(dst, src)
                    dst0 = t0.rearrange("p (w s j) -> p w s j", s=2, j=32)[
                        :, q * 16:(q + 1) * 16]
                    nc.scalar.mul(dst0, src, ukw[rt][:, 0:1])
                y.append(ysb)
                t0s.append(t0)

            # --- conv_w (VectorE + tmp from ACT drains) ---
            z = []
            for rt in range(2):
                zt = zpool.tile([128, PLANE], bf16, name="zw", tag="z")
                # z = U1*y
                nc.vector.tensor_scalar_mul(zt, y[rt], ukw[rt][:, 1:2])
                zv = zt.rearrange("p (w q) -> p w q", q=64)
                t0v = t0s[rt].rearrange("p (w q) -> p w q", q=64)
                yv = y[rt].rearrange("p (w q) -> p w q", q=64)
                # z[w] += t0[w-1]
                nc.vector.tensor_tensor(zv[:, 1:, :], t0v[:, :-1, :], zv[:, 1:, :], add)
                # t2 = U2*y ; z[w] += t2[w+1]
                t2 = tpool.tile([128, PLANE], bf16, name="t2t", tag="tmp")
                nc.vector.tensor_scalar_mul(t2, y[rt], ukw[rt][:, 2:3])
                t2v = t2.rearrange("p (w q) -> p w q", q=64)
                nc.vector.tensor_tensor(zv[:, :-1, :], t2v[:, 1:, :], zv[:, :-1, :], add)
                z.append(zt)

            # --- conv_d (VectorE scales, adds split DVE/GpSimd) ---
            zd = []
            for rt in range(2):
                zt = zdpool.tile([128, PLANE], bf16, name="zdt", tag="zd")
                a0 = tpool.tile([128, PLANE], bf16, name="a0t", tag="tmp")
                a2 = tpool.tile([128, PLANE], bf16, name="a2t", tag="tmp")
                nc.vector.tensor_scalar_mul(zt, z[rt], ukd[rt][:, 1:2])
                nc.vector.tensor_scalar_mul(a0, z[rt], ukd[rt][:, 0:1])
                nc.vector.tensor_scalar_mul(a2, z[rt], ukd[rt][:, 2:3])
                zv = zt.rearrange("p (w s j) -> p w s j", s=2, j=32)
                a0v = a0.rearrange("p (w s j) -> p w s j", s=2, j=32)
                a2v = a2.rearrange("p (w s j) -> p w s j", s=2, j=32)
                eng = nc.vector if rt == 0 else nc.gpsimd
                # even outputs d=2j:  += a0[2j-1] (j>=1), += a2[2j+1]
                eng.tensor_tensor(zv[:, :, 0, 1:], a0v[:, :, 1, :-1], zv[:, :, 0, 1:], add)
                eng.tensor_tensor(zv[:, :, 0, :], a2v[:, :, 1, :], zv[:, :, 0, :], add)
                # odd outputs d=2j+1: += a0[2j], += a2[2j+2] (j<=30)
                eng.tensor_tensor(zv[:, :, 1, :], a0v[:, :, 0, :], zv[:, :, 1, :], add)
                eng.tensor_tensor(zv[:, :, 1, :-1], a2v[:, :, 0, 1:], zv[:, :, 1, :-1], add)
                zd.append(zt)

            # --- mm2 + bias drain ---
            for co in range(2):
                osb = opool.tile([128, PLANE], fp32, name="osb", tag="osb")
                for q in range(NQ):
                    pt = ps2.tile([128, 1024], fp32, name="pt2", tag="ps2")
                    for half in range(2):
                        for rt in range(2):
                            nc.tensor.matmul(
                                pt[:, half * 512:(half + 1) * 512],
                                uco[rt][co],
                                zd[rt][:, q * 1024 + half * 512:
                                       q * 1024 + (half + 1) * 512],
                                start=(rt == 0),
                                stop=(rt == 1),
                            )
                    # drain with bias, re-interleave d
                    dst = osb.rearrange("p (w j s) -> p w s j", j=32, s=2)[
                        :, q * 16:(q + 1) * 16]
                    src = pt.rearrange("p (w s j) -> p w s j", s=2, j=32)
                    nc.scalar.activation(dst, src, ident, bias=bia[co][:, 0:1])
                nc.sync.dma_start(out=out_d[co, :, h, :], in_=osb)

        for pool in (ps2, ps1, opool, zdpool, zpool, tpool, ypool, xpool, consts):
            pool.release()

    nc.compile()
    return nc


def _host_prep(x, U_kh, U_kw, U_kd, U_c_in, U_c_out, bias):
    """Build per-core input maps (numpy only)."""
    x = np.asarray(x)
    U_kh = np.asarray(U_kh, np.float32)
    U_kw = np.asarray(U_kw, np.float32)
    U_kd = np.asarray(U_kd, np.float32)
    U_c_in = np.asarray(U_c_in, np.float32)
    U_c_out = np.asarray(U_c_out, np.float32)
    bias = np.asarray(bias, np.float32)

    xb = np.ascontiguousarray(x[0]).astype(BF16)          # [C, S, S, S]
    xb = xb.reshape(C, S, PLANE)

    # W_k[c, r] = U_c_in[c,r] * U_kh[k,r]  -> [3, ct, rt, 128, 128]
    wkh = np.empty((3, 2, 2, 128, 128), BF16)
    for k in range(3):
        wk = (U_c_in * U_kh[k][None, :]).astype(BF16)     # [C, R]
        wkh[k] = wk.reshape(2, 128, 2, 128).transpose(0, 2, 1, 3)

    uco = U_c_out.astype(BF16).reshape(2, 128, 2, 128).transpose(0, 2, 1, 3)
    uco = np.ascontiguousarray(uco)
    ukw = np.ascontiguousarray(U_kw.T.reshape(2, 128, 3))
    ukd = np.ascontiguousarray(U_kd.T.reshape(2, 128, 3))
    bias_t = np.ascontiguousarray(bias.reshape(2, 128, 1))

    in_maps = []
    for c in range(NCORES):
        slab = np.zeros((C, HS, PLANE), BF16)
        lo, hi = c * HP - 1, c * HP + HP + 1
        s0, s1 = max(lo, 0), min(hi, S)
        slab[:, s0 - lo:HS - (hi - s1)] = xb[:, s0:s1]
        slab = np.ascontiguousarray(slab.reshape(2, 128, HS, PLANE))
        in_maps.append({
            "xs": slab, "wkh": wkh, "uco": uco, "ukw": ukw,
            "ukd": ukd, "bias_t": bias_t,
        })
    return in_maps


def kernel(x, U_kh, U_kw, U_kd, U_c_in, U_c_out, bias, _trace=False):
    from concourse.bass_utils import run_bass_kernel_spmd

    if "nc" not in _cache:
        _cache["nc"] = _build_program()
    nc = _cache["nc"]

    in_maps = _host_prep(x, U_kh, U_kw, U_kd, U_c_in, U_c_out, bias)
    res = run_bass_kernel_spmd(nc, in_maps, core_ids=list(range(NCORES)),
                               trace=_trace)
    _cache["last_result"] = res

    out = np.empty((1, CO, S, S, S), np.float32)
    for c in range(NCORES):
        o = res.results[c]["out"]                        # [2, 128, HP, PLANE]
        out[0, :, c * HP:(c + 1) * HP] = o.reshape(CO, HP, S, S)
    return out



# revision 2
# speedup vs baseline: 1.0465x; 1.0465x over previous
"""Low-rank (CPD) 3D conv kernel for Trainium2, SPMD across 8 NeuronCores.

Per-core pipeline (H split 8x8 with 1-plane halo, no collectives):
  mm1 (PE): y'' = sum_{c,kh} (U_c_in*U_kh[kh]*U_kw[1]*U_kd[1])^T x(h+kh)
            -- conv_h folded via 3 h-tap weights, center-tap scales of
            conv_w and conv_d pre-folded into the weights.
  conv_w (DVE): in-place on the drained tile using ratio-scaled temps:
            t0 = u*(U_kw[0]/U_kw[1]), t2 = u*(U_kw[2]/U_kw[1]);
            u[64:] += t0[:-64]; u[:-64] += t2[64:]
  conv_d (DVE + GpSimd): d de-interleaved on host (evens|odds per w-line);
            4 compact half-plane temps with U_kd ratios; the two aligned
            adds on DVE (2x mode), the two misaligned on GpSimd.
  mm2 (PE): out[co] = sum_rt U_c_out^T u + bias (drains on ACT, bf16 out).

PE stream: weight-stationary groups (4 matmuls per LDWEIGHTS candidate),
single shared PSUM pool 4 x [128,1024] = 8 banks, mm2 lagging one plane so
the elementwise chain hides behind mm1 of the next plane. LDWEIGHTS dedup
via walrus --enable-ldw-opt.
"""

import numpy as np
import ml_dtypes

BF16 = ml_dtypes.bfloat16

C = 256
R = 256
CO = 256
S = 64
NCORES = 8
HP = S // NCORES
HS = HP + 2
PLANE = S * S

_cache = {}


def _patch_ldw_dedup():
    """Post-legalize pass: drop InstLdweights whose weights AP equals the
    previously loaded one (weights are already resident in the PE array),
    transferring their dependency edges to the next matmul. The emission
    order pins consecutive same-weight matmul runs via add_dep_helper, so
    this is deterministic."""
    import concourse.tile as tile
    if getattr(tile, "_ldw_dedup_patched", False):
        return
    orig = tile.tile_legalize

    def dedup(ordered, nc):
        out = orig(ordered, nc)
        for bbname, insts in out.items():
            newl = []
            cur_w = None
            pending = None
            for inst in insts:
                tn = type(inst).__name__
                if tn == 'InstLdweights':
                    sig = str(inst.ins[0])
                    if sig == cur_w:
                        if pending is not None:
                            inst.merge_dependencies_from(pending)
                        pending = inst
                        continue
                    cur_w = sig
                if pending is not None:
                    inst.merge_dependencies_from(pending)
                    pending = None
                newl.append(inst)
            assert pending is None
            out[bbname] = newl
        return out

    tile.tile_legalize = dedup
    tile._ldw_dedup_patched = True


def _build_program(hp=HP):
    import concourse.mybir as mybir
    import concourse.tile as tile
    from concourse import bacc
    from concourse.tile_rust import add_dep_helper

    _patch_ldw_dedup()

    HS_, HP_ = hp + 2, hp

    fp32 = mybir.dt.float32
    bf16 = mybir.dt.bfloat16

    nc = bacc.Bacc("TRN2", target_bir_lowering=False, debug=False,
                   num_devices=NCORES)

    x_d = nc.dram_tensor("xs", [2, 128, HS_, PLANE], bf16, kind="ExternalInput").ap()
    wkh_d = nc.dram_tensor("wkh", [3, 2, 2, 128, 128], bf16, kind="ExternalInput").ap()
    uco_d = nc.dram_tensor("uco", [2, 2, 128, 128], bf16, kind="ExternalInput").ap()
    # ratio taps: [rt, 128, 2] = (tap0, tap2) / clamped center tap
    rw_d = nc.dram_tensor("rw", [2, 128, 2], fp32, kind="ExternalInput").ap()
    rd_d = nc.dram_tensor("rd", [2, 128, 2], fp32, kind="ExternalInput").ap()
    bias_d = nc.dram_tensor("bias_t", [2, 128, 1], fp32, kind="ExternalInput").ap()
    out_d = nc.dram_tensor("out", [2, 128, HP_, PLANE], bf16, kind="ExternalOutput").ap()

    add = mybir.AluOpType.add
    ident = mybir.ActivationFunctionType.Identity

    with tile.TileContext(nc) as tc:
        consts = tc.alloc_tile_pool(name="consts", bufs=1)
        xpool = tc.alloc_tile_pool(name="x", bufs=8)
        upool = tc.alloc_tile_pool(name="u", bufs=5)
        twpool = tc.alloc_tile_pool(name="tw", bufs=2)
        tdpool = tc.alloc_tile_pool(name="td", bufs=2)
        opool = tc.alloc_tile_pool(name="osb", bufs=2)
        pspool = tc.alloc_tile_pool(name="ps", bufs=4, space="PSUM")

        wkh = [[[consts.tile([128, 128], bf16, name=f"wkh{k}{ct}{rt}", tag=f"wkh{k}{ct}{rt}")
                 for rt in range(2)] for ct in range(2)] for k in range(3)]
        for k in range(3):
            for ct in range(2):
                for rt in range(2):
                    nc.sync.dma_start(out=wkh[k][ct][rt], in_=wkh_d[k, ct, rt])
        uco = [[consts.tile([128, 128], bf16, name=f"uco{rt}{co}", tag=f"uco{rt}{co}")
                for co in range(2)] for rt in range(2)]
        for rt in range(2):
            for co in range(2):
                nc.sync.dma_start(out=uco[rt][co], in_=uco_d[rt, co])
        rw = [consts.tile([128, 2], fp32, name=f"rw{rt}", tag=f"rw{rt}") for rt in range(2)]
        rd = [consts.tile([128, 2], fp32, name=f"rd{rt}", tag=f"rd{rt}") for rt in range(2)]
        bia = [consts.tile([128, 1], fp32, name=f"bias{co}", tag=f"bias{co}") for co in range(2)]
        for rt in range(2):
            nc.sync.dma_start(out=rw[rt], in_=rw_d[rt])
            nc.sync.dma_start(out=rd[rt], in_=rd_d[rt])
        for co in range(2):
            nc.sync.dma_start(out=bia[co], in_=bias_d[co])

        xt = {}

        def get_x(p, ct):
            if (p, ct) not in xt:
                t = xpool.tile([128, PLANE], bf16, name="xplane", tag="xplane")
                nc.sync.dma_start(out=t, in_=x_d[ct, :, p, :])
                xt[(p, ct)] = t
            return xt[(p, ct)]

        def mm_group(psum_tiles, spans, weights):
            """Weight-stationary accumulation with pinned PE ordering so the
            walrus ldw-opt sees runs of identical consecutive weights."""
            nw = len(weights)
            prev = None
            for wi, (lhsT, rhs_list) in enumerate(weights):
                idx = 0
                for t in psum_tiles:
                    for (lo, hi) in spans:
                        mm = nc.tensor.matmul(
                            t[:, lo:hi], lhsT, rhs_list[idx],
                            start=(wi == 0), stop=(wi == nw - 1),
                        )
                        if prev is not None:
                            add_dep_helper(mm.ins, prev.ins, sync=False,
                                           reason="pin PE order for ldw dedup")
                        prev = mm
                        idx += 1
            return prev

        def emit_mm2(h, z):
            for co in range(2):
                osb = opool.tile([128, PLANE], bf16, name="osb", tag="osb")
                for g in range(2):
                    base = g * 2048
                    tA = pspool.tile([128, 1024], fp32, name="ps", tag="ps")
                    tB = pspool.tile([128, 1024], fp32, name="ps", tag="ps")
                    spans = [(0, 512), (512, 1024)]
                    weights = []
                    for rt in range(2):
                        rhs_list = []
                        for ti in range(2):
                            off = base + ti * 1024
                            rhs_list.append(z[rt][:, off:off + 512])
                            rhs_list.append(z[rt][:, off + 512:off + 1024])
                        weights.append((uco[rt][co], rhs_list))
                    mm_group((tA, tB), spans, weights)
                    # mm2 drains on ACT with bias add, bf16 out
                    nc.scalar.activation(osb[:, base:base + 1024], tA, ident,
                                         bias=bia[co][:, 0:1])
                    nc.scalar.activation(osb[:, base + 1024:base + 2048], tB, ident,
                                         bias=bia[co][:, 0:1])
                nc.sync.dma_start(out=out_d[co, :, h, :], in_=osb)

        # ---- PE warmup: ~16 dummy matmuls on memset tiles so the HAM clock
        # gate reaches K=8/8 while the first x planes stream in ----
        wsb = consts.tile([128, 128], bf16, name="wsb", tag="wsb")
        xsb = consts.tile([128, 512], bf16, name="xsb", tag="xsb")
        nc.gpsimd.memset(wsb, 0.0)
        nc.gpsimd.memset(xsb, 0.0)
        wu = pspool.tile([128, 1024], fp32, name="ps", tag="ps")
        prev_wu = None
        for _ in range(16):
            mm = nc.tensor.matmul(wu[:, 0:512], wsb, xsb, start=True, stop=True)
            if prev_wu is not None:
                add_dep_helper(mm.ins, prev_wu.ins, sync=False, reason="warmup order")
            prev_wu = mm

        z_prev = None
        for h in range(HP_):
            # ---- mm1: conv_h + center-tap scales folded ----
            u = []
            for rt in range(2):
                ut = upool.tile([128, PLANE], bf16, name="ut", tag="u")
                for g in range(2):
                    base = g * 2048
                    tA = pspool.tile([128, 1024], fp32, name="ps", tag="ps")
                    tB = pspool.tile([128, 1024], fp32, name="ps", tag="ps")
                    spans = [(0, 512), (512, 1024)]
                    weights = []
                    for k in range(3):
                        for ct in range(2):
                            xp = get_x(h + k, ct)
                            rhs_list = []
                            for ti in range(2):
                                off = base + ti * 1024
                                rhs_list.append(xp[:, off:off + 512])
                                rhs_list.append(xp[:, off + 512:off + 1024])
                            weights.append((wkh[k][ct][rt], rhs_list))
                    mm_group((tA, tB), spans, weights)
                    # mm1 drains on ACT: f32 psum -> bf16
                    nc.scalar.copy(ut[:, base:base + 1024], tA)
                    nc.scalar.copy(ut[:, base + 1024:base + 2048], tB)
                u.append(ut)

            # ---- conv_w in-place (ratio temps on DVE) ----
            for rt in range(2):
                ut = u[rt]
                t0 = twpool.tile([128, PLANE], bf16, name="t0", tag="tw0")
                t2 = twpool.tile([128, PLANE], bf16, name="t2", tag="tw2")
                nc.vector.tensor_scalar_mul(t0, ut, rw[rt][:, 0:1])
                nc.vector.tensor_scalar_mul(t2, ut, rw[rt][:, 1:2])
                nc.vector.tensor_tensor(ut[:, 64:], t0[:, :PLANE - 64], ut[:, 64:], add)
                nc.vector.tensor_tensor(ut[:, :PLANE - 64], t2[:, 64:], ut[:, :PLANE - 64], add)

            # ---- conv_d in-place. The shift-by-one temps are produced on ACT
            # (no accel modes to lose there), already shifted into aligned
            # position with a zeroed boundary column, so every add is an
            # aligned 2x tensor_tensor. GpSimd only gets the early B-adds so
            # it never tails the chain. ----
            for rt in range(2):
                ut = u[rt]
                u3 = ut.rearrange("p (w c) -> p w c", c=64)
                # temps from pristine u halves (evens 0:32, odds 32:64)
                toU2 = tdpool.tile([128, PLANE // 2], bf16, name="toU2", tag="toU2")
                teU0 = tdpool.tile([128, PLANE // 2], bf16, name="teU0", tag="teU0")
                t0s = tdpool.tile([128, PLANE // 2], bf16, name="t0s", tag="t0s")
                t2s = tdpool.tile([128, PLANE // 2], bf16, name="t2s", tag="t2s")
                toU2v = toU2.rearrange("p (w j) -> p w j", j=32)
                teU0v = teU0.rearrange("p (w j) -> p w j", j=32)
                t0sv = t0s.rearrange("p (w j) -> p w j", j=32)
                t2sv = t2s.rearrange("p (w j) -> p w j", j=32)
                nc.vector.tensor_scalar_mul(toU2, u3[:, :, 32:64], rd[rt][:, 1:2])
                nc.vector.tensor_scalar_mul(teU0, u3[:, :, 0:32], rd[rt][:, 0:1])
                # shifted temps on ACT: t0s[w,j] = U0d*u_o[w,j-1] (j>=1),
                # t2s[w,j] = U2d*u_e[w,j+1] (j<=30); boundary cols zeroed.
                nc.scalar.mul(t0sv[:, :, 1:32], u3[:, :, 32:63], rd[rt][:, 0:1])
                nc.scalar.mul(t2sv[:, :, 0:31], u3[:, :, 1:32], rd[rt][:, 1:2])
                nc.gpsimd.memset(t0sv[:, :, 0:1], 0.0)
                nc.gpsimd.memset(t2sv[:, :, 31:32], 0.0)
                # B: u_e[j] += U2d*u_o[j]   (DVE 2x)
                nc.vector.tensor_tensor(u3[:, :, 0:32], toU2v[:, :, 0:32],
                                        u3[:, :, 0:32], add)
                # C: u_o[j] += U0d*u_e[j]   (DVE 2x)
                nc.vector.tensor_tensor(u3[:, :, 32:64], teU0v[:, :, 0:32],
                                        u3[:, :, 32:64], add)
                # A: u_e += t0s   (aligned now -> DVE 2x)
                nc.vector.tensor_tensor(u3[:, :, 0:32], t0sv[:, :, 0:32],
                                        u3[:, :, 0:32], add)
                # D: u_o += t2s   (aligned now -> DVE 2x)
                nc.vector.tensor_tensor(u3[:, :, 32:64], t2sv[:, :, 0:32],
                                        u3[:, :, 32:64], add)

            if h > 0:
                emit_mm2(h - 1, z_prev)
            z_prev = u
        emit_mm2(HP_ - 1, z_prev)

        for pool in (pspool, opool, tdpool, twpool, upool, xpool, consts):
            pool.release()

    nc.compile()
    return nc


def _host_prep(x, U_kh, U_kw, U_kd, U_c_in, U_c_out, bias):
    x = np.asarray(x)
    U_kh = np.asarray(U_kh, np.float32)
    U_kw = np.asarray(U_kw, np.float32)
    U_kd = np.asarray(U_kd, np.float32)
    U_c_in = np.asarray(U_c_in, np.float32)
    U_c_out = np.asarray(U_c_out, np.float32)
    bias = np.asarray(bias, np.float32)

    # de-interleave d on host: col = w*64 + (d%2)*32 + d//2
    xv = np.asarray(x[0]).reshape(C, S, S, 32, 2)
    xb = np.ascontiguousarray(xv.transpose(0, 1, 2, 4, 3)).astype(BF16)
    xb = xb.reshape(C, S, PLANE)

    # clamp center taps away from zero so the ratios stay finite
    eps = np.float32(1e-6)
    cw = U_kw[1].copy()
    cw[np.abs(cw) < eps] = eps
    cd = U_kd[1].copy()
    cd[np.abs(cd) < eps] = eps

    # mm1 weights: U_c_in * U_kh[k] * cw * cd   (center taps pre-folded)
    wkh = np.empty((3, 2, 2, 128, 128), BF16)
    for k in range(3):
        wk = (U_c_in * (U_kh[k] * cw * cd)[None, :]).astype(BF16)
        wkh[k] = wk.reshape(2, 128, 2, 128).transpose(0, 2, 1, 3)

    uco = U_c_out.astype(BF16).reshape(2, 128, 2, 128).transpose(0, 2, 1, 3)
    uco = np.ascontiguousarray(uco)
    rw = np.ascontiguousarray(
        np.stack([U_kw[0] / cw, U_kw[2] / cw], axis=1).reshape(2, 128, 2))
    rdm = np.ascontiguousarray(
        np.stack([U_kd[0] / cd, U_kd[2] / cd], axis=1).reshape(2, 128, 2))
    bias_t = np.ascontiguousarray(bias.reshape(2, 128, 1))

    in_maps = []
    for c in range(NCORES):
        slab = np.zeros((C, HS, PLANE), BF16)
        lo, hi = c * HP - 1, c * HP + HP + 1
        s0, s1 = max(lo, 0), min(hi, S)
        slab[:, s0 - lo:HS - (hi - s1)] = xb[:, s0:s1]
        slab = np.ascontiguousarray(slab.reshape(2, 128, HS, PLANE))
        in_maps.append({
            "xs": slab, "wkh": wkh, "uco": uco, "rw": rw,
            "rd": rdm, "bias_t": bias_t,
        })
    return in_maps


def kernel(x, U_kh, U_kw, U_kd, U_c_in, U_c_out, bias, _trace=False):
    from concourse.bass_utils import run_bass_kernel_spmd

    if "nc" not in _cache:
        _cache["nc"] = _build_program()
    nc = _cache["nc"]

    in_maps = _host_prep(x, U_kh, U_kw, U_kd, U_c_in, U_c_out, bias)
    res = run_bass_kernel_spmd(nc, in_maps, core_ids=list(range(NCORES)),
                               trace=_trace)
    _cache["last_result"] = res

    out = np.empty((1, CO, S, S, S), np.float32)
    for c in range(NCORES):
        o = res.results[c]["out"]
        ov = np.asarray(o).astype(np.float32).reshape(CO, HP, S, 2, 32)
        out[0, :, c * HP:(c + 1) * HP] = ov.transpose(0, 1, 2, 4, 3).reshape(
            CO, HP, S, S)
    return out


# revision 3
# speedup vs baseline: 1.0582x; 1.0112x over previous
"""Low-rank (CPD) 3D conv kernel for Trainium2, SPMD across 8 NeuronCores. v3.

Per-core pipeline (H split 8x8 with 1-plane halo, no collectives):
  mm1 (PE): y'' = sum_{c,kh} (U_c_in*U_kh[kh]*U_kw[1]*U_kd[1])^T x(h+kh)
            -- conv_h folded via 3 h-tap weights, center-tap scales of
            conv_w and conv_d pre-folded into the weights.
  conv_w (DVE): in-place on the drained tile using ratio-scaled temps:
            t0 = u*(U_kw[0]/U_kw[1]), t2 = u*(U_kw[2]/U_kw[1]);
            u[64:] += t0[:-64]; u[:-64] += t2[64:]
  conv_d (DVE + GpSimd): d de-interleaved on host (evens|odds per w-line);
            4 compact half-plane temps with U_kd ratios; the two aligned
            adds on DVE (2x mode), the two misaligned on GpSimd.
  mm2 (PE): out[co] = sum_rt U_c_out^T u + bias (drains on ACT, bf16 out).

PE stream: weight-stationary groups (4 matmuls per LDWEIGHTS candidate),
single shared PSUM pool 4 x [128,1024] = 8 banks, mm2 lagging one plane so
the elementwise chain hides behind mm1 of the next plane. LDWEIGHTS dedup
via walrus --enable-ldw-opt.
"""

import numpy as np
import ml_dtypes

BF16 = ml_dtypes.bfloat16

C = 256
R = 256
CO = 256
S = 64
NCORES = 8
HP = S // NCORES
HS = HP + 2
PLANE = S * S

_cache = {}


def _patch_ldw_dedup():
    """Post-legalize pass: drop InstLdweights whose weights AP equals the
    previously loaded one (weights are already resident in the PE array),
    transferring their dependency edges to the next matmul. The emission
    order pins consecutive same-weight matmul runs via add_dep_helper, so
    this is deterministic."""
    import concourse.tile as tile
    if getattr(tile, "_ldw_dedup_patched", False):
        return
    orig = tile.tile_legalize

    def dedup(ordered, nc):
        out = orig(ordered, nc)
        for bbname, insts in out.items():
            newl = []
            cur_w = None
            pending = None
            for inst in insts:
                tn = type(inst).__name__
                if tn == 'InstLdweights':
                    sig = str(inst.ins[0])
                    if sig == cur_w:
                        if pending is not None:
                            inst.merge_dependencies_from(pending)
                        pending = inst
                        continue
                    cur_w = sig
                if pending is not None:
                    inst.merge_dependencies_from(pending)
                    pending = None
                newl.append(inst)
            assert pending is None
            out[bbname] = newl
        return out

    tile.tile_legalize = dedup
    tile._ldw_dedup_patched = True


def _build_program(hp=HP):
    import concourse.mybir as mybir
    import concourse.tile as tile
    from concourse import bacc
    from concourse.tile_rust import add_dep_helper

    _patch_ldw_dedup()

    HS_, HP_ = hp + 2, hp

    fp32 = mybir.dt.float32
    bf16 = mybir.dt.bfloat16

    nc = bacc.Bacc("TRN2", target_bir_lowering=False, debug=False,
                   num_devices=NCORES)

    x_d = nc.dram_tensor("xs", [2, 128, HS_, PLANE], bf16, kind="ExternalInput").ap()
    wkh_d = nc.dram_tensor("wkh", [3, 2, 2, 128, 128], bf16, kind="ExternalInput").ap()
    uco_d = nc.dram_tensor("uco", [2, 2, 128, 128], bf16, kind="ExternalInput").ap()
    # ratio taps: [rt, 128, 2] = (tap0, tap2) / clamped center tap
    rw_d = nc.dram_tensor("rw", [2, 128, 2], fp32, kind="ExternalInput").ap()
    rd_d = nc.dram_tensor("rd", [2, 128, 2], fp32, kind="ExternalInput").ap()
    bias_d = nc.dram_tensor("bias_t", [2, 128, 1], fp32, kind="ExternalInput").ap()
    out_d = nc.dram_tensor("out", [2, 128, HP_, PLANE], bf16, kind="ExternalOutput").ap()

    add = mybir.AluOpType.add
    ident = mybir.ActivationFunctionType.Identity

    with tile.TileContext(nc) as tc:
        consts = tc.alloc_tile_pool(name="consts", bufs=1)
        xpool = tc.alloc_tile_pool(name="x", bufs=8)
        upool = tc.alloc_tile_pool(name="u", bufs=5)
        twpool = tc.alloc_tile_pool(name="tw", bufs=2)
        tdpool = tc.alloc_tile_pool(name="td", bufs=2)
        opool = tc.alloc_tile_pool(name="osb", bufs=2)
        pspool = tc.alloc_tile_pool(name="ps", bufs=4, space="PSUM")

        wkh = [[[consts.tile([128, 128], bf16, name=f"wkh{k}{ct}{rt}", tag=f"wkh{k}{ct}{rt}")
                 for rt in range(2)] for ct in range(2)] for k in range(3)]
        for k in range(3):
            for ct in range(2):
                for rt in range(2):
                    nc.sync.dma_start(out=wkh[k][ct][rt], in_=wkh_d[k, ct, rt])
        uco = [[consts.tile([128, 128], bf16, name=f"uco{rt}{co}", tag=f"uco{rt}{co}")
                for co in range(2)] for rt in range(2)]
        for rt in range(2):
            for co in range(2):
                nc.sync.dma_start(out=uco[rt][co], in_=uco_d[rt, co])
        rw = [consts.tile([128, 2], fp32, name=f"rw{rt}", tag=f"rw{rt}") for rt in range(2)]
        rd = [consts.tile([128, 2], fp32, name=f"rd{rt}", tag=f"rd{rt}") for rt in range(2)]
        bia = [consts.tile([128, 1], fp32, name=f"bias{co}", tag=f"bias{co}") for co in range(2)]
        for rt in range(2):
            nc.sync.dma_start(out=rw[rt], in_=rw_d[rt])
            nc.sync.dma_start(out=rd[rt], in_=rd_d[rt])
        for co in range(2):
            nc.sync.dma_start(out=bia[co], in_=bias_d[co])

        xt = {}

        def get_x(p, ct):
            if (p, ct) not in xt:
                t = xpool.tile([128, PLANE], bf16, name="xplane", tag="xplane")
                nc.sync.dma_start(out=t, in_=x_d[ct, :, p, :])
                xt[(p, ct)] = t
            return xt[(p, ct)]

        def mm_group(psum_tiles, spans, weights):
            """Weight-stationary accumulation with pinned PE ordering so the
            walrus ldw-opt sees runs of identical consecutive weights."""
            nw = len(weights)
            prev = None
            for wi, (lhsT, rhs_list) in enumerate(weights):
                idx = 0
                for t in psum_tiles:
                    for (lo, hi) in spans:
                        mm = nc.tensor.matmul(
                            t[:, lo:hi], lhsT, rhs_list[idx],
                            start=(wi == 0), stop=(wi == nw - 1),
                        )
                        if prev is not None:
                            add_dep_helper(mm.ins, prev.ins, sync=False,
                                           reason="pin PE order for ldw dedup")
                        prev = mm
                        idx += 1
            return prev

        def emit_mm2(h, z):
            for co in range(2):
                osb = opool.tile([128, PLANE], bf16, name="osb", tag="osb")
                for g in range(2):
                    base = g * 2048
                    tA = pspool.tile([128, 1024], fp32, name="ps", tag="ps")
                    tB = pspool.tile([128, 1024], fp32, name="ps", tag="ps")
                    spans = [(0, 512), (512, 1024)]
                    weights = []
                    for rt in range(2):
                        rhs_list = []
                        for ti in range(2):
                            off = base + ti * 1024
                            rhs_list.append(z[rt][:, off:off + 512])
                            rhs_list.append(z[rt][:, off + 512:off + 1024])
                        weights.append((uco[rt][co], rhs_list))
                    mm_group((tA, tB), spans, weights)
                    # mm2 drains on ACT with bias add, bf16 out
                    nc.scalar.activation(osb[:, base:base + 1024], tA, ident,
                                         bias=bia[co][:, 0:1])
                    nc.scalar.activation(osb[:, base + 1024:base + 2048], tB, ident,
                                         bias=bia[co][:, 0:1])
                nc.sync.dma_start(out=out_d[co, :, h, :], in_=osb)

        # ---- PE warmup: ~16 dummy matmuls on memset tiles so the HAM clock
        # gate reaches K=8/8 while the first x planes stream in ----
        wsb = consts.tile([128, 128], bf16, name="wsb", tag="wsb")
        xsb = consts.tile([128, 512], bf16, name="xsb", tag="xsb")
        tick = consts.tile([128, 512], bf16, name="tick", tag="tick")
        nc.gpsimd.memset(wsb, 0.0)
        nc.gpsimd.memset(xsb, 0.0)
        # drip-feed the warmup: a WAW-chained run of GpSimd memsets acts as a
        # slow clock; each 3-matmul warmup burst is gated on one tick, so the
        # PE sees a burst every ~1.5-2us across the whole x-DMA ramp and the
        # HAM gate stays open without touching the x tiles.
        ticks = [nc.gpsimd.memset(tick, 0.0) for _ in range(12)]
        wu = pspool.tile([128, 1024], fp32, name="ps", tag="ps")
        prev_wu = None
        for gi in range(12):
            for mi in range(3):
                mm = nc.tensor.matmul(wu[:, 0:512], wsb, xsb, start=True, stop=True)
                if mi == 0:
                    add_dep_helper(mm.ins, ticks[gi].ins, sync=True,
                                   reason="warmup drip gate")
                if prev_wu is not None:
                    add_dep_helper(mm.ins, prev_wu.ins, sync=False,
                                   reason="warmup order")
                prev_wu = mm

        z_prev = None
        for h in range(HP_):
            # ---- mm1: conv_h + center-tap scales folded ----
            u = []
            for rt in range(2):
                ut = upool.tile([128, PLANE], bf16, name="ut", tag="u")
                for g in range(2):
                    base = g * 2048
                    tA = pspool.tile([128, 1024], fp32, name="ps", tag="ps")
                    tB = pspool.tile([128, 1024], fp32, name="ps", tag="ps")
                    spans = [(0, 512), (512, 1024)]
                    weights = []
                    for k in range(3):
                        for ct in range(2):
                            xp = get_x(h + k, ct)
                            rhs_list = []
                            for ti in range(2):
                                off = base + ti * 1024
                                rhs_list.append(xp[:, off:off + 512])
                                rhs_list.append(xp[:, off + 512:off + 1024])
                            weights.append((wkh[k][ct][rt], rhs_list))
                    mm_group((tA, tB), spans, weights)
                    # mm1 drains on ACT: f32 psum -> bf16
                    nc.scalar.copy(ut[:, base:base + 1024], tA)
                    nc.scalar.copy(ut[:, base + 1024:base + 2048], tB)
                u.append(ut)

            # ---- conv_w in-place (ratio temps on DVE) ----
            for rt in range(2):
                ut = u[rt]
                t0 = twpool.tile([128, PLANE], bf16, name="t0", tag="tw0")
                t2 = twpool.tile([128, PLANE], bf16, name="t2", tag="tw2")
                nc.vector.tensor_scalar_mul(t0, ut, rw[rt][:, 0:1])
                nc.vector.tensor_scalar_mul(t2, ut, rw[rt][:, 1:2])
                nc.vector.tensor_tensor(ut[:, 64:], t0[:, :PLANE - 64], ut[:, 64:], add)
                nc.vector.tensor_tensor(ut[:, :PLANE - 64], t2[:, 64:], ut[:, :PLANE - 64], add)

            if h > 0:
                emit_mm2(h - 1, z_prev)

            # ---- conv_d in-place. The shift-by-one temps are produced on ACT
            # (no accel modes to lose there), already shifted into aligned
            # position with a zeroed boundary column, so every add is an
            # aligned 2x tensor_tensor. GpSimd only gets the early B-adds so
            # it never tails the chain. ----
            for rt in range(2):
                ut = u[rt]
                u3 = ut.rearrange("p (w c) -> p w c", c=64)
                # temps from pristine u halves (evens 0:32, odds 32:64)
                toU2 = tdpool.tile([128, PLANE // 2], bf16, name="toU2", tag="toU2")
                teU0 = tdpool.tile([128, PLANE // 2], bf16, name="teU0", tag="teU0")
                t0s = tdpool.tile([128, PLANE // 2], bf16, name="t0s", tag="t0s")
                t2s = tdpool.tile([128, PLANE // 2], bf16, name="t2s", tag="t2s")
                toU2v = toU2.rearrange("p (w j) -> p w j", j=32)
                teU0v = teU0.rearrange("p (w j) -> p w j", j=32)
                t0sv = t0s.rearrange("p (w j) -> p w j", j=32)
                t2sv = t2s.rearrange("p (w j) -> p w j", j=32)
                nc.vector.tensor_scalar_mul(toU2, u3[:, :, 32:64], rd[rt][:, 1:2])
                nc.vector.tensor_scalar_mul(teU0, u3[:, :, 0:32], rd[rt][:, 0:1])
                # shifted temps on ACT: t0s[w,j] = U0d*u_o[w,j-1] (j>=1),
                # t2s[w,j] = U2d*u_e[w,j+1] (j<=30); boundary cols zeroed.
                nc.scalar.mul(t0sv[:, :, 1:32], u3[:, :, 32:63], rd[rt][:, 0:1])
                nc.scalar.mul(t2sv[:, :, 0:31], u3[:, :, 1:32], rd[rt][:, 1:2])
                nc.gpsimd.memset(t0sv[:, :, 0:1], 0.0)
                nc.gpsimd.memset(t2sv[:, :, 31:32], 0.0)
                # B: u_e[j] += U2d*u_o[j]   (DVE 2x)
                nc.vector.tensor_tensor(u3[:, :, 0:32], toU2v[:, :, 0:32],
                                        u3[:, :, 0:32], add)
                # C: u_o[j] += U0d*u_e[j]   (DVE 2x)
                nc.vector.tensor_tensor(u3[:, :, 32:64], teU0v[:, :, 0:32],
                                        u3[:, :, 32:64], add)
                # A: u_e += t0s   (aligned now -> DVE 2x)
                nc.vector.tensor_tensor(u3[:, :, 0:32], t0sv[:, :, 0:32],
                                        u3[:, :, 0:32], add)
                # D: u_o += t2s   (aligned now -> DVE 2x)
                nc.vector.tensor_tensor(u3[:, :, 32:64], t2sv[:, :, 0:32],
                                        u3[:, :, 32:64], add)

            z_prev = u
        emit_mm2(HP_ - 1, z_prev)

        for pool in (pspool, opool, tdpool, twpool, upool, xpool, consts):
            pool.release()

    nc.compile()
    return nc


def _host_prep(x, U_kh, U_kw, U_kd, U_c_in, U_c_out, bias):
    x = np.asarray(x)
    U_kh = np.asarray(U_kh, np.float32)
    U_kw = np.asarray(U_kw, np.float32)
    U_kd = np.asarray(U_kd, np.float32)
    U_c_in = np.asarray(U_c_in, np.float32)
    U_c_out = np.asarray(U_c_out, np.float32)
    bias = np.asarray(bias, np.float32)

    # de-interleave d on host: col = w*64 + (d%2)*32 + d//2
    xv = np.asarray(x[0]).reshape(C, S, S, 32, 2)
    xb = np.ascontiguousarray(xv.transpose(0, 1, 2, 4, 3)).astype(BF16)
    xb = xb.reshape(C, S, PLANE)

    # clamp center taps away from zero so the ratios stay finite
    eps = np.float32(1e-6)
    cw = U_kw[1].copy()
    cw[np.abs(cw) < eps] = eps
    cd = U_kd[1].copy()
    cd[np.abs(cd) < eps] = eps

    # mm1 weights: U_c_in * U_kh[k] * cw * cd   (center taps pre-folded)
    wkh = np.empty((3, 2, 2, 128, 128), BF16)
    for k in range(3):
        wk = (U_c_in * (U_kh[k] * cw * cd)[None, :]).astype(BF16)
        wkh[k] = wk.reshape(2, 128, 2, 128).transpose(0, 2, 1, 3)

    uco = U_c_out.astype(BF16).reshape(2, 128, 2, 128).transpose(0, 2, 1, 3)
    uco = np.ascontiguousarray(uco)
    rw = np.ascontiguousarray(
        np.stack([U_kw[0] / cw, U_kw[2] / cw], axis=1).reshape(2, 128, 2))
    rdm = np.ascontiguousarray(
        np.stack([U_kd[0] / cd, U_kd[2] / cd], axis=1).reshape(2, 128, 2))
    bias_t = np.ascontiguousarray(bias.reshape(2, 128, 1))

    in_maps = []
    for c in range(NCORES):
        slab = np.zeros((C, HS, PLANE), BF16)
        lo, hi = c * HP - 1, c * HP + HP + 1
        s0, s1 = max(lo, 0), min(hi, S)
        slab[:, s0 - lo:HS - (hi - s1)] = xb[:, s0:s1]
        slab = np.ascontiguousarray(slab.reshape(2, 128, HS, PLANE))
        in_maps.append({
            "xs": slab, "wkh": wkh, "uco": uco, "rw": rw,
            "rd": rdm, "bias_t": bias_t,
        })
    return in_maps


def kernel(x, U_kh, U_kw, U_kd, U_c_in, U_c_out, bias, _trace=False):
    from concourse.bass_utils import run_bass_kernel_spmd

    if "nc" not in _cache:
        _cache["nc"] = _build_program()
    nc = _cache["nc"]

    in_maps = _host_prep(x, U_kh, U_kw, U_kd, U_c_in, U_c_out, bias)
    res = run_bass_kernel_spmd(nc, in_maps, core_ids=list(range(NCORES)),
                               trace=_trace)
    _cache["last_result"] = res

    out = np.empty((1, CO, S, S, S), np.float32)
    for c in range(NCORES):
        o = res.results[c]["out"]
        ov = np.asarray(o).astype(np.float32).reshape(CO, HP, S, 2, 32)
        out[0, :, c * HP:(c + 1) * HP] = ov.transpose(0, 1, 2, 4, 3).reshape(
            CO, HP, S, S)
    return out


# revision 4
# speedup vs baseline: 1.0760x; 1.0168x over previous
"""Low-rank (CPD) 3D conv kernel for Trainium2, SPMD across 8 NeuronCores.

Per-core pipeline (H split 8x8 with 1-plane halo, no collectives):
  mm1 (PE): y'' = sum_{c,kh} (U_c_in*U_kh[kh]*U_kw[1]*U_kd[1])^T x(h+kh)
            -- conv_h folded via 3 h-tap weights, center-tap scales of
            conv_w and conv_d pre-folded into the weights.
  conv_w (DVE): in-place on the drained tile using ratio-scaled temps:
            t0 = u*(U_kw[0]/U_kw[1]), t2 = u*(U_kw[2]/U_kw[1]);
            u[64:] += t0[:-64]; u[:-64] += t2[64:]
  conv_d (DVE + GpSimd): d de-interleaved on host (evens|odds per w-line);
            4 compact half-plane temps with U_kd ratios; the two aligned
            adds on DVE (2x mode), the two misaligned on GpSimd.
  mm2 (PE): out[co] = sum_rt U_c_out^T u + bias (drains on ACT, bf16 out).

PE stream: weight-stationary matmul groups with a post-legalize pass that
drops redundant InstLdweights (weights already resident in the PE array),
single shared PSUM pool 4 x [128,1024] = 8 banks, mm2 lagging one plane so
the elementwise chain hides behind mm1 of the next plane, a drip-fed PE
warmup so the HAM clock gate stays at K=8/8 through the initial DMA ramp,
and emission ordered so psum-freeing ACT drains never queue behind
lower-priority ACT work.
"""

import numpy as np
import ml_dtypes

BF16 = ml_dtypes.bfloat16

C = 256
R = 256
CO = 256
S = 64
NCORES = 8
HP = S // NCORES
HS = HP + 2
PLANE = S * S

_cache = {}


def _patch_ldw_dedup():
    """Post-legalize pass: drop InstLdweights whose weights AP equals the
    previously loaded one (weights are already resident in the PE array),
    transferring their dependency edges to the next matmul. The emission
    order pins consecutive same-weight matmul runs via add_dep_helper, so
    this is deterministic."""
    import concourse.tile as tile
    if getattr(tile, "_ldw_dedup_patched", False):
        return
    orig = tile.tile_legalize

    def dedup(ordered, nc):
        out = orig(ordered, nc)
        for bbname, insts in out.items():
            newl = []
            cur_w = None
            pending = None
            for inst in insts:
                tn = type(inst).__name__
                if tn == 'InstLdweights':
                    sig = str(inst.ins[0])
                    if sig == cur_w:
                        if pending is not None:
                            inst.merge_dependencies_from(pending)
                        pending = inst
                        continue
                    cur_w = sig
                if pending is not None:
                    inst.merge_dependencies_from(pending)
                    pending = None
                newl.append(inst)
            assert pending is None
            out[bbname] = newl
        return out

    tile.tile_legalize = dedup
    tile._ldw_dedup_patched = True


def _build_program(hp=HP):
    import concourse.mybir as mybir
    import concourse.tile as tile
    from concourse import bacc
    from concourse.tile_rust import add_dep_helper

    _patch_ldw_dedup()

    HS_, HP_ = hp + 2, hp

    fp32 = mybir.dt.float32
    bf16 = mybir.dt.bfloat16

    nc = bacc.Bacc("TRN2", target_bir_lowering=False, debug=False,
                   num_devices=NCORES)

    x_d = nc.dram_tensor("xs", [2, 128, HS_, PLANE], bf16, kind="ExternalInput").ap()
    wkh_d = nc.dram_tensor("wkh", [3, 2, 2, 128, 128], bf16, kind="ExternalInput").ap()
    uco_d = nc.dram_tensor("uco", [2, 2, 128, 128], bf16, kind="ExternalInput").ap()
    # ratio taps: [rt, 128, 2] = (tap0, tap2) / clamped center tap
    rw_d = nc.dram_tensor("rw", [2, 128, 2], fp32, kind="ExternalInput").ap()
    rd_d = nc.dram_tensor("rd", [2, 128, 2], fp32, kind="ExternalInput").ap()
    bias_d = nc.dram_tensor("bias_t", [2, 128, 1], fp32, kind="ExternalInput").ap()
    out_d = nc.dram_tensor("out", [2, 128, HP_, PLANE], bf16, kind="ExternalOutput").ap()

    add = mybir.AluOpType.add
    ident = mybir.ActivationFunctionType.Identity

    with tile.TileContext(nc) as tc:
        consts = tc.alloc_tile_pool(name="consts", bufs=1)
        xpool = tc.alloc_tile_pool(name="x", bufs=8)
        upool = tc.alloc_tile_pool(name="u", bufs=5)
        twpool = tc.alloc_tile_pool(name="tw", bufs=2)
        tdpool = tc.alloc_tile_pool(name="td", bufs=2)
        opool = tc.alloc_tile_pool(name="osb", bufs=2)
        pspool = tc.alloc_tile_pool(name="ps", bufs=4, space="PSUM")

        wkh = [[[consts.tile([128, 128], bf16, name=f"wkh{k}{ct}{rt}", tag=f"wkh{k}{ct}{rt}")
                 for rt in range(2)] for ct in range(2)] for k in range(3)]
        for k in range(3):
            for ct in range(2):
                for rt in range(2):
                    nc.sync.dma_start(out=wkh[k][ct][rt], in_=wkh_d[k, ct, rt])
        uco = [[consts.tile([128, 128], bf16, name=f"uco{rt}{co}", tag=f"uco{rt}{co}")
                for co in range(2)] for rt in range(2)]
        for rt in range(2):
            for co in range(2):
                nc.sync.dma_start(out=uco[rt][co], in_=uco_d[rt, co])
        rw = [consts.tile([128, 2], fp32, name=f"rw{rt}", tag=f"rw{rt}") for rt in range(2)]
        rd = [consts.tile([128, 2], fp32, name=f"rd{rt}", tag=f"rd{rt}") for rt in range(2)]
        bia = [consts.tile([128, 1], fp32, name=f"bias{co}", tag=f"bias{co}") for co in range(2)]
        for rt in range(2):
            nc.sync.dma_start(out=rw[rt], in_=rw_d[rt])
            nc.sync.dma_start(out=rd[rt], in_=rd_d[rt])
        for co in range(2):
            nc.sync.dma_start(out=bia[co], in_=bias_d[co])

        xt = {}

        def get_x(p, ct):
            if (p, ct) not in xt:
                t = xpool.tile([128, PLANE], bf16, name="xplane", tag="xplane")
                nc.sync.dma_start(out=t, in_=x_d[ct, :, p, :])
                xt[(p, ct)] = t
            return xt[(p, ct)]

        def mm_group(psum_tiles, spans, weights):
            """Weight-stationary accumulation with pinned PE ordering so the
            walrus ldw-opt sees runs of identical consecutive weights."""
            nw = len(weights)
            prev = None
            for wi, (lhsT, rhs_list) in enumerate(weights):
                idx = 0
                for t in psum_tiles:
                    for (lo, hi) in spans:
                        mm = nc.tensor.matmul(
                            t[:, lo:hi], lhsT, rhs_list[idx],
                            start=(wi == 0), stop=(wi == nw - 1),
                        )
                        if prev is not None:
                            add_dep_helper(mm.ins, prev.ins, sync=False,
                                           reason="pin PE order for ldw dedup")
                        prev = mm
                        idx += 1
            return prev

        def emit_mm2(h, z):
            for co in range(2):
                osb = opool.tile([128, PLANE], bf16, name="osb", tag="osb")
                for g in range(2):
                    base = g * 2048
                    tA = pspool.tile([128, 1024], fp32, name="ps", tag="ps")
                    tB = pspool.tile([128, 1024], fp32, name="ps", tag="ps")
                    spans = [(0, 512), (512, 1024)]
                    weights = []
                    for rt in range(2):
                        rhs_list = []
                        for ti in range(2):
                            off = base + ti * 1024
                            rhs_list.append(z[rt][:, off:off + 512])
                            rhs_list.append(z[rt][:, off + 512:off + 1024])
                        weights.append((uco[rt][co], rhs_list))
                    mm_group((tA, tB), spans, weights)
                    # mm2 drains on ACT with bias add, bf16 out
                    nc.scalar.activation(osb[:, base:base + 1024], tA, ident,
                                         bias=bia[co][:, 0:1])
                    nc.scalar.activation(osb[:, base + 1024:base + 2048], tB, ident,
                                         bias=bia[co][:, 0:1])
                nc.sync.dma_start(out=out_d[co, :, h, :], in_=osb)

        # ---- PE warmup: ~16 dummy matmuls on memset tiles so the HAM clock
        # gate reaches K=8/8 while the first x planes stream in ----
        wsb = consts.tile([128, 128], bf16, name="wsb", tag="wsb")
        xsb = consts.tile([128, 512], bf16, name="xsb", tag="xsb")
        tick = consts.tile([128, 512], bf16, name="tick", tag="tick")
        nc.gpsimd.memset(wsb, 0.0)
        nc.gpsimd.memset(xsb, 0.0)
        # drip-feed the warmup: a WAW-chained run of GpSimd memsets acts as a
        # slow clock; each 3-matmul warmup burst is gated on one tick, so the
        # PE sees a burst every ~1.5-2us across the whole x-DMA ramp and the
        # HAM gate stays open without touching the x tiles.
        ticks = [nc.gpsimd.memset(tick, 0.0) for _ in range(12)]
        wu = pspool.tile([128, 1024], fp32, name="ps", tag="ps")
        prev_wu = None
        for gi in range(12):
            for mi in range(3):
                mm = nc.tensor.matmul(wu[:, 0:512], wsb, xsb, start=True, stop=True)
                if mi == 0:
                    add_dep_helper(mm.ins, ticks[gi].ins, sync=True,
                                   reason="warmup drip gate")
                if prev_wu is not None:
                    add_dep_helper(mm.ins, prev_wu.ins, sync=False,
                                   reason="warmup order")
                prev_wu = mm

        z_prev = None
        for h in range(HP_):
            # ---- mm1: conv_h + center-tap scales folded ----
            u = []
            for rt in range(2):
                ut = upool.tile([128, PLANE], bf16, name="ut", tag="u")
                for g in range(2):
                    base = g * 2048
                    tA = pspool.tile([128, 1024], fp32, name="ps", tag="ps")
                    tB = pspool.tile([128, 1024], fp32, name="ps", tag="ps")
                    spans = [(0, 512), (512, 1024)]
                    weights = []
                    for k in range(3):
                        for ct in range(2):
                            xp = get_x(h + k, ct)
                            rhs_list = []
                            for ti in range(2):
                                off = base + ti * 1024
                                rhs_list.append(xp[:, off:off + 512])
                                rhs_list.append(xp[:, off + 512:off + 1024])
                            weights.append((wkh[k][ct][rt], rhs_list))
                    mm_group((tA, tB), spans, weights)
                    # mm1 drains on ACT: f32 psum -> bf16
                    nc.scalar.copy(ut[:, base:base + 1024], tA)
                    nc.scalar.copy(ut[:, base + 1024:base + 2048], tB)
                u.append(ut)

            # ---- conv_w in-place (ratio temps on DVE) ----
            for rt in range(2):
                ut = u[rt]
                t0 = twpool.tile([128, PLANE], bf16, name="t0", tag="tw0")
                t2 = twpool.tile([128, PLANE], bf16, name="t2", tag="tw2")
                nc.vector.tensor_scalar_mul(t0, ut, rw[rt][:, 0:1])
                nc.vector.tensor_scalar_mul(t2, ut, rw[rt][:, 1:2])
                nc.vector.tensor_tensor(ut[:, 64:], t0[:, :PLANE - 64], ut[:, 64:], add)
                nc.vector.tensor_tensor(ut[:, :PLANE - 64], t2[:, 64:], ut[:, :PLANE - 64], add)

            if 0 < h < HP_ - 1:
                emit_mm2(h - 1, z_prev)

            # ---- conv_d in-place. The shift-by-one temps are produced on ACT
            # (no accel modes to lose there), already shifted into aligned
            # position with a zeroed boundary column, so every add is an
            # aligned 2x tensor_tensor. GpSimd only gets the early B-adds so
            # it never tails the chain. ----
            for rt in range(2):
                ut = u[rt]
                u3 = ut.rearrange("p (w c) -> p w c", c=64)
                # temps from pristine u halves (evens 0:32, odds 32:64)
                toU2 = tdpool.tile([128, PLANE // 2], bf16, name="toU2", tag="toU2")
                teU0 = tdpool.tile([128, PLANE // 2], bf16, name="teU0", tag="teU0")
                t0s = tdpool.tile([128, PLANE // 2], bf16, name="t0s", tag="t0s")
                t2s = tdpool.tile([128, PLANE // 2], bf16, name="t2s", tag="t2s")
                toU2v = toU2.rearrange("p (w j) -> p w j", j=32)
                teU0v = teU0.rearrange("p (w j) -> p w j", j=32)
                t0sv = t0s.rearrange("p (w j) -> p w j", j=32)
                t2sv = t2s.rearrange("p (w j) -> p w j", j=32)
                nc.vector.tensor_scalar_mul(toU2, u3[:, :, 32:64], rd[rt][:, 1:2])
                nc.vector.tensor_scalar_mul(teU0, u3[:, :, 0:32], rd[rt][:, 0:1])
                # shifted temps on ACT: t0s[w,j] = U0d*u_o[w,j-1] (j>=1),
                # t2s[w,j] = U2d*u_e[w,j+1] (j<=30); boundary cols zeroed.
                nc.scalar.mul(t0sv[:, :, 1:32], u3[:, :, 32:63], rd[rt][:, 0:1])
                nc.scalar.mul(t2sv[:, :, 0:31], u3[:, :, 1:32], rd[rt][:, 1:2])
                nc.gpsimd.memset(t0sv[:, :, 0:1], 0.0)
                nc.gpsimd.memset(t2sv[:, :, 31:32], 0.0)
                # B: u_e[j] += U2d*u_o[j]   (DVE 2x)
                nc.vector.tensor_tensor(u3[:, :, 0:32], toU2v[:, :, 0:32],
                                        u3[:, :, 0:32], add)
                # C: u_o[j] += U0d*u_e[j]   (DVE 2x)
                nc.vector.tensor_tensor(u3[:, :, 32:64], teU0v[:, :, 0:32],
                                        u3[:, :, 32:64], add)
                # A: u_e += t0s   (aligned now -> DVE 2x)
                nc.vector.tensor_tensor(u3[:, :, 0:32], t0sv[:, :, 0:32],
                                        u3[:, :, 0:32], add)
                # D: u_o += t2s   (aligned now -> DVE 2x)
                nc.vector.tensor_tensor(u3[:, :, 32:64], t2sv[:, :, 0:32],
                                        u3[:, :, 32:64], add)

            if h == HP_ - 1:
                # last plane: its ACT shifted-temps are data-ready before
                # mm2(h-1)'s drains, so emit them first to shorten the tail
                emit_mm2(h - 1, z_prev)
            z_prev = u
        emit_mm2(HP_ - 1, z_prev)

        for pool in (pspool, opool, tdpool, twpool, upool, xpool, consts):
            pool.release()

    nc.compile()
    return nc


def _host_prep(x, U_kh, U_kw, U_kd, U_c_in, U_c_out, bias):
    x = np.asarray(x)
    U_kh = np.asarray(U_kh, np.float32)
    U_kw = np.asarray(U_kw, np.float32)
    U_kd = np.asarray(U_kd, np.float32)
    U_c_in = np.asarray(U_c_in, np.float32)
    U_c_out = np.asarray(U_c_out, np.float32)
    bias = np.asarray(bias, np.float32)

    # de-interleave d on host: col = w*64 + (d%2)*32 + d//2
    xv = np.asarray(x[0]).reshape(C, S, S, 32, 2)
    xb = np.ascontiguousarray(xv.transpose(0, 1, 2, 4, 3)).astype(BF16)
    xb = xb.reshape(C, S, PLANE)

    # clamp center taps away from zero so the ratios stay finite
    eps = np.float32(1e-6)
    cw = U_kw[1].copy()
    cw[np.abs(cw) < eps] = eps
    cd = U_kd[1].copy()
    cd[np.abs(cd) < eps] = eps

    # mm1 weights: U_c_in * U_kh[k] * cw * cd   (center taps pre-folded)
    wkh = np.empty((3, 2, 2, 128, 128), BF16)
    for k in range(3):
        wk = (U_c_in * (U_kh[k] * cw * cd)[None, :]).astype(BF16)
        wkh[k] = wk.reshape(2, 128, 2, 128).transpose(0, 2, 1, 3)

    uco = U_c_out.astype(BF16).reshape(2, 128, 2, 128).transpose(0, 2, 1, 3)
    uco = np.ascontiguousarray(uco)
    rw = np.ascontiguousarray(
        np.stack([U_kw[0] / cw, U_kw[2] / cw], axis=1).reshape(2, 128, 2))
    rdm = np.ascontiguousarray(
        np.stack([U_kd[0] / cd, U_kd[2] / cd], axis=1).reshape(2, 128, 2))
    bias_t = np.ascontiguousarray(bias.reshape(2, 128, 1))

    in_maps = []
    for c in range(NCORES):
        slab = np.zeros((C, HS, PLANE), BF16)
        lo, hi = c * HP - 1, c * HP + HP + 1
        s0, s1 = max(lo, 0), min(hi, S)
        slab[:, s0 - lo:HS - (hi - s1)] = xb[:, s0:s1]
        slab = np.ascontiguousarray(slab.reshape(2, 128, HS, PLANE))
        in_maps.append({
            "xs": slab, "wkh": wkh, "uco": uco, "rw": rw,
            "rd": rdm, "bias_t": bias_t,
        })
    return in_maps


def kernel(x, U_kh, U_kw, U_kd, U_c_in, U_c_out, bias, _trace=False):
    from concourse.bass_utils import run_bass_kernel_spmd

    if "nc" not in _cache:
        _cache["nc"] = _build_program()
    nc = _cache["nc"]

    in_maps = _host_prep(x, U_kh, U_kw, U_kd, U_c_in, U_c_out, bias)
    res = run_bass_kernel_spmd(nc, in_maps, core_ids=list(range(NCORES)),
                               trace=_trace)
    _cache["last_result"] = res

    out = np.empty((1, CO, S, S, S), np.float32)
    for c in range(NCORES):
        o = res.results[c]["out"]
        ov = np.asarray(o).astype(np.float32).reshape(CO, HP, S, 2, 32)
        out[0, :, c * HP:(c + 1) * HP] = ov.transpose(0, 1, 2, 4, 3).reshape(
            CO, HP, S, S)
    return out
